# revision 31
# baseline (speedup 1.0000x reference)
"""Trainium2 Bass kernel for nn_Attn_48137993453608.

Module: Y = X@W1.T+b1 -> split Q,K,V -> w = softmax((Q_h^T K_h)/sqrt(S))
        (attention over the DH=64 dim, contracting S) -> out = w @ V_h^T
        -> raw memory-order reshape [B,H,DH,S]->[B,S,D] -> @ W2.T + b2.

Sharding: 8 cores = 4 batch x 2 head-groups (8 heads each). Each core owns a
contiguous [1024, 1024] block of the output (rows i = 128*h + 2*d + (s>=1024)
for its heads), so no collectives are needed.

Per-core dataflow (all on-chip after the initial loads):
  1. Yqk[s, n]  = Xb @ Wqk.T + bqk    (n: 512 Q cols | 512 K cols, local heads)
  2. VT[vrow,s] = Wv @ Xb.T + bv      (v rows for local heads)
  3. wT_h[e, d] = sum_s K_h[s,e] Q_h[s,d]   (PSUM accum over all s)
  4. expwT = exp(wT / sqrt(S))        (no max-sub: |logits| <= ~6)
  5. per head pair: block-diag expwT2 [128,128]; Z = column sums via
     ones-matmul; rZ = 1/Z
  6. OT_un[s, c2] = VT2^T @ expwT2    ([2048, 128] per pair)
  7. F[c2, n] = (sum_j OT_un[j,c2] W2T[j,n]) * rZ[c2] + b2[n]  per (pair, half)
  8. scatter F rows to the output block: r = 256*p + 128*g + 2*d + half
"""

import os
import sys

for _p in ("/opt/trn_rl_repo",):
    if _p not in sys.path and os.path.isdir(_p):
        sys.path.insert(0, _p)

import numpy as np

import concourse.bass as bass
import concourse.bacc as bacc
import concourse.mybir as mybir
import concourse.tile as tile
from concourse.bass_utils import run_bass_kernel_spmd

B, S, D, H = 4, 2048, 1024, 16
DH = D // H          # 64
NH = 8               # heads per core
SCALE = 1.0 / float(np.sqrt(np.float32(S)))

F32 = mybir.dt.float32
F32R = mybir.dt.float32r

S_CHUNK = 256                 # s columns of X^T staged per iteration
N_SCHUNKS = S // S_CHUNK      # 8
ST_PER_CHUNK = S_CHUNK // 128 # 2


def build_nc():
    nc = bacc.Bacc("TRN2", target_bir_lowering=False, debug=False)

    # --- per-core DRAM I/O (float32r is bit-identical to f32; enables
    # 1-cycle/row matmuls at free dim >= 256) ---
    xbt = nc.dram_tensor("xbt", [D, S], F32R, kind="ExternalInput")        # X[b].T
    wqkt = nc.dram_tensor("wqkt", [D, 1024], F32R, kind="ExternalInput")
    wvt = nc.dram_tensor("wvt", [D, 512], F32R, kind="ExternalInput")
    bqk = nc.dram_tensor("bqk", [1, 1024], F32, kind="ExternalInput")
    bvt = nc.dram_tensor("bvt", [128, 4], F32, kind="ExternalInput")       # bv.reshape(4,128).T
    w2t = nc.dram_tensor("w2t", [D, 1024], F32R, kind="ExternalInput")     # W2.T
    b2 = nc.dram_tensor("b2", [1, 1024], F32, kind="ExternalInput")
    out = nc.dram_tensor("out", [1024, 1024], F32, kind="ExternalOutput")

    # DRAM views with the 128-partition dim innermost-major for SBUF loads
    xbt_v = xbt[:].rearrange("(kb p) s -> p kb s", p=128)      # [128, 8, 2048]
    wvt_v = wvt[:].rearrange("(kb p) n -> p kb n", p=128)      # [128, 8, 512]
    w2t_v = w2t[:].rearrange("(jb p) n -> p jb n", p=128)      # [128, 8, 1024]
    # output rows r = 256*p + 128*g + 2*d + half
    out_v = out[:].rearrange("(p g d h) n -> p g d h n", p=4, g=2, d=64, h=2)

    with tile.TileContext(nc) as tc:
        with (
            tc.tile_pool(name="const", bufs=1) as const,
            tc.tile_pool(name="xin", bufs=3) as xin,
            tc.tile_pool(name="ywork", bufs=3) as ywork,
            tc.tile_pool(name="vtp", bufs=1) as vtp,
            tc.tile_pool(name="attn", bufs=1) as attn,
            tc.tile_pool(name="otp", bufs=2) as otp,
            tc.tile_pool(name="fout", bufs=2) as fout,
            tc.tile_pool(name="psacc", bufs=3, space="PSUM") as psacc,
            tc.tile_pool(name="pswt", bufs=1, space="PSUM") as pswt,
            tc.tile_pool(name="psot", bufs=2, space="PSUM") as psot,
            tc.tile_pool(name="psf", bufs=2, space="PSUM") as psf,
        ):
            # ---------------- phase-1 loads (phase-3 loads deferred) ------
            # The first QK matmul needs xbt chunk0 + wqk kb=0 only: submit
            # those first so the saturable (~25GB/s/engine) DMA engines serve
            # the critical path before the bulk weight traffic.
            xbt_tiles = []
            xbt_sb0 = xin.tile([128, 8, S_CHUNK], F32R, tag="xbt")
            nc.sync.dma_start(out=xbt_sb0[:], in_=xbt_v[:, :, 0:S_CHUNK])
            xbt_tiles.append(xbt_sb0)

            # Per-kb loads keep 4KB contiguous bursts and let the first QK
            # matmul start after only the kb=0 block (512KB) has landed.
            wqk_sb = const.tile([128, 8, 1024], F32R)
            for kb in range(8):
                nc.scalar.dma_start(out=wqk_sb[:, kb, :],
                                    in_=wqkt[kb * 128:(kb + 1) * 128, :])
            wv_sb = const.tile([128, 8, 512], F32R)
            for kb in range(0, 8, 2):
                nc.scalar.dma_start(out=wv_sb[:, kb:kb + 2, :],
                                    in_=wvt_v[:, kb:kb + 2, :])

            bqk_bc = const.tile([128, 1024], F32)
            nc.gpsimd.dma_start(out=bqk_bc[:], in_=bqk[:].to_broadcast((128, 1024)))
            bv_sb = const.tile([128, 4], F32)
            nc.gpsimd.dma_start(out=bv_sb[:], in_=bvt[:])

            ones_sb = const.tile([128, 1], F32)
            nc.vector.memset(ones_sb[:], 1.0)

            # VT persists until the OT matmuls; wT accumulates across all s.
            vt_sb = vtp.tile([128, 4, S], F32R)         # [e2, pair, s]
            psum_wt = pswt.tile([128, 512], F32)        # [e2(g*64+e), pair*128+c2]

            # ---------------- phase 1: QK, VT, wT ----------------
            for sc in range(N_SCHUNKS):
                if sc < len(xbt_tiles):
                    xbt_sb = xbt_tiles[sc]
                else:
                    xbt_sb = xin.tile([128, 8, S_CHUNK], F32R, tag="xbt")
                    nc.sync.dma_start(
                        out=xbt_sb[:],
                        in_=xbt_v[:, :, sc * S_CHUNK:(sc + 1) * S_CHUNK],
                    )

                yqk_tiles = []
                for st in range(ST_PER_CHUNK):
                    s_lo = st * 128
                    yqk_sb = ywork.tile([128, 1024], F32, tag="yqk")
                    yqk_tiles.append(yqk_sb)
                    for nh in range(2):
                        ps_y = psacc.tile([128, 512], F32, tag="acc")
                        for kb in range(8):
                            nc.tensor.matmul(
                                ps_y[:],
                                lhsT=xbt_sb[:, kb, s_lo:s_lo + 128],
                                rhs=wqk_sb[:, kb, nh * 512:(nh + 1) * 512],
                                start=(kb == 0),
                                stop=(kb == 7),
                            )
                        nc.vector.tensor_tensor(
                            out=yqk_sb[:, nh * 512:(nh + 1) * 512],
                            in0=ps_y[:],
                            in1=bqk_bc[:, nh * 512:(nh + 1) * 512],
                            op=mybir.AluOpType.add,
                        )

                # VT: out [vrow-block, s-chunk].  Traced between the QK
                # matmuls and the wT matmuls so the DVE evictions of yqk
                # have a full VT window to drain before PE needs them
                # (avoids head-of-line stalls on the in-order PE queue).
                for mb in range(4):
                    ps_v = psacc.tile([128, S_CHUNK], F32, tag="acc")
                    for kb in range(8):
                        nc.tensor.matmul(
                            ps_v[:],
                            lhsT=wv_sb[:, kb, mb * 128:(mb + 1) * 128],
                            rhs=xbt_sb[:, kb, :],
                            start=(kb == 0),
                            stop=(kb == 7),
                        )
                    nc.vector.tensor_scalar_add(
                        vt_sb[:, mb, sc * S_CHUNK:(sc + 1) * S_CHUNK],
                        ps_v[:],
                        bv_sb[:, mb:mb + 1],
                    )

                # wT accumulation: lhsT=K_h slice, rhs=Q_h slice
                for st in range(ST_PER_CHUNK):
                    yqk_sb = yqk_tiles[st]
                    for hl in range(NH):
                        p, g = hl // 2, hl % 2
                        nc.tensor.matmul(
                            psum_wt[g * 64:(g + 1) * 64,
                                    p * 128 + g * 64:p * 128 + (g + 1) * 64],
                            lhsT=yqk_sb[:, 512 + hl * 64:512 + (hl + 1) * 64],
                            rhs=yqk_sb[:, hl * 64:(hl + 1) * 64],
                            # start=True clears has_written for the WHOLE bank
                            # row of the written partitions -> only the first
                            # matmul per partition-half may set it.
                            start=(sc == 0 and st == 0 and hl < 2),
                            stop=(sc == N_SCHUNKS - 1 and st == ST_PER_CHUNK - 1),
                            skip_group_check=True,
                        )

            # ---------------- phase-3 weights (overlap with phase 1) ------
            w2_sb = const.tile([128, 8, 1024], F32R)
            nc.scalar.dma_start(out=w2_sb[:], in_=w2t_v)
            b2_bc = const.tile([128, 1024], F32)
            nc.gpsimd.dma_start(out=b2_bc[:], in_=b2[:].to_broadcast((128, 1024)))

            # ---------------- phase 2: exp, Z ----------------
            expw_f32 = attn.tile([128, 4, 128], F32)
            nc.vector.memset(expw_f32[:], 0.0)
            for hl in range(NH):
                p, g = hl // 2, hl % 2
                nc.scalar.activation(
                    out=expw_f32[g * 64:(g + 1) * 64, p, g * 64:(g + 1) * 64],
                    in_=psum_wt[g * 64:(g + 1) * 64,
                                p * 128 + g * 64:p * 128 + (g + 1) * 64],
                    func=mybir.ActivationFunctionType.Exp,
                    scale=SCALE,
                )
            expw_sb = attn.tile([128, 4, 128], F32R)
            nc.vector.tensor_copy(expw_sb[:], expw_f32[:])
            ps_z = psacc.tile([128, 4], F32, tag="acc")
            rz_sb = attn.tile([128, 4], F32)
            for p in range(4):
                nc.tensor.matmul(
                    ps_z[:, p:p + 1],
                    lhsT=expw_f32[:, p, :],
                    rhs=ones_sb[:],
                    start=(p == 0),
                    stop=(p == 3),
                    skip_group_check=True,
                )
            nc.vector.reciprocal(rz_sb[:], ps_z[:])

            # ---------------- phase 3: OT, F, store ----------------
            for p in range(4):
                ot_sb = otp.tile([128, 16, 128], F32R, tag="ot")
                for sbq in range(4):
                    ps_ot = psot.tile([128, 4, 128], F32, tag="psot")
                    for i in range(4):
                        nc.tensor.matmul(
                            ps_ot[:, i, :],
                            lhsT=vt_sb[:, p, (sbq * 4 + i) * 128:(sbq * 4 + i + 1) * 128],
                            rhs=expw_sb[:, p, :],
                            start=(i == 0),
                            stop=(i == 3),
                            skip_group_check=True,
                        )
                    nc.scalar.copy(ot_sb[:, sbq * 4:(sbq + 1) * 4, :], ps_ot[:])
                for half in range(2):
                    f_sb = fout.tile([128, 1024], F32, tag="f")
                    for nh in range(2):
                        ps_f = psf.tile([128, 512], F32, tag="psf")
                        for sb8 in range(8):
                            nc.tensor.matmul(
                                ps_f[:],
                                lhsT=ot_sb[:, half * 8 + sb8, :],
                                rhs=w2_sb[:, sb8, nh * 512:(nh + 1) * 512],
                                start=(sb8 == 0),
                                stop=(sb8 == 7),
                            )
                        # F = psum * rZ (per partition) + b2
                        nc.vector.scalar_tensor_tensor(
                            out=f_sb[:, nh * 512:(nh + 1) * 512],
                            in0=ps_f[:],
                            scalar=rz_sb[:, p:p + 1],
                            in1=b2_bc[:, nh * 512:(nh + 1) * 512],
                            op0=mybir.AluOpType.mult,
                            op1=mybir.AluOpType.add,
                        )
                    nc.sync.dma_start(out=out_v[p, :, :, half, :], in_=f_sb[:])

    nc.finalize()
    return nc


_NC_CACHE = None


def _get_nc():
    global _NC_CACHE
    if _NC_CACHE is None:
        _NC_CACHE = build_nc()
    return _NC_CACHE


def _shard_inputs(X, W1, b1, W2, b2):
    X = np.asarray(X, np.float32)
    W1 = np.asarray(W1, np.float32)
    b1 = np.asarray(b1, np.float32)
    W2 = np.asarray(W2, np.float32)
    b2 = np.asarray(b2, np.float32)

    w2t = np.ascontiguousarray(W2.T)
    b2r = np.ascontiguousarray(b2.reshape(1, 1024))
    xbts = [np.ascontiguousarray(X[b].T) for b in range(B)]

    per_hg = []
    for hg in range(2):
        heads = range(NH * hg, NH * hg + NH)
        qrows = np.concatenate(
            [np.arange(h * DH, (h + 1) * DH) for h in heads]
            + [D + np.arange(h * DH, (h + 1) * DH) for h in heads])
        vrows = np.concatenate(
            [2 * D + np.arange(h * DH, (h + 1) * DH) for h in heads])
        wqkt = np.ascontiguousarray(W1[qrows].T)
        bqk = np.ascontiguousarray(b1[qrows].reshape(1, 1024))
        wvt = np.ascontiguousarray(W1[vrows].T)
        bvt = np.ascontiguousarray(b1[vrows].reshape(4, 128).T)
        per_hg.append((wqkt, bqk, wvt, bvt))

    in_maps = []
    for c in range(8):
        b, hg = c // 2, c % 2
        wqkt, bqk, wvt, bvt = per_hg[hg]
        in_maps.append({
            "xbt": xbts[b], "wqkt": wqkt, "wvt": wvt, "bqk": bqk,
            "bvt": bvt, "w2t": w2t, "b2": b2r,
        })
    return in_maps


def run(X, W1, b1, W2, b2, **run_kwargs):
    """Returns (full_output, BassKernelResults)."""
    nc = _get_nc()
    in_maps = _shard_inputs(X, W1, b1, W2, b2)
    res = run_bass_kernel_spmd(nc, in_maps, core_ids=list(range(8)), **run_kwargs)
    full = np.empty((B, S, D), np.float32)
    for c in range(8):
        b, hg = c // 2, c % 2
        full[b, hg * 1024:(hg + 1) * 1024, :] = res.results[c]["out"]
    return full, res


def kernel(X, W1, b1, W2, b2):
    return run(X, W1, b1, W2, b2)[0]


# revision 35
# speedup vs baseline: 1.1566x; 1.1566x over previous
"""Trainium2 Bass kernel for nn_Attn_48137993453608.

Module: Y = X@W1.T+b1 -> split Q,K,V -> w = softmax((Q_h^T K_h)/sqrt(S))
        (attention over the DH=64 dim, contracting S) -> out = w @ V_h^T
        -> raw memory-order reshape [B,H,DH,S]->[B,S,D] -> @ W2.T + b2.

Sharding: 8 cores = 4 batch x 2 head-groups (8 heads each). Each core owns a
contiguous [1024, 1024] block of the output (rows i = 128*h + 2*d + (s>=1024)
for its heads), so no collectives are needed.

Per-core dataflow (all on-chip after the initial loads):
  1. Yqk[s, n]  = Xb @ Wqk.T + bqk    (n: 512 Q cols | 512 K cols, local heads)
  2. VT[vrow,s] = Wv @ Xb.T + bv      (v rows for local heads)
  3. wT_h[e, d] = sum_s K_h[s,e] Q_h[s,d]   (PSUM accum over all s)
  4. expwT = exp(wT / sqrt(S))        (no max-sub: |logits| <= ~6)
  5. per head pair: block-diag expwT2 [128,128]; Z = column sums via
     ones-matmul; rZ = 1/Z
  6. OT_un[s, c2] = VT2^T @ expwT2    ([2048, 128] per pair)
  7. F[c2, n] = (sum_j OT_un[j,c2] W2T[j,n]) * rZ[c2] + b2[n]  per (pair, half)
  8. scatter F rows to the output block: r = 256*p + 128*g + 2*d + half
"""

import os
import sys

for _p in ("/opt/trn_rl_repo",):
    if _p not in sys.path and os.path.isdir(_p):
        sys.path.insert(0, _p)

import ml_dtypes
import numpy as np

import concourse.bass as bass
import concourse.bacc as bacc
import concourse.mybir as mybir
import concourse.tile as tile
from concourse.bass_utils import run_bass_kernel_spmd

B, S, D, H = 4, 2048, 1024, 16
DH = D // H          # 64
NH = 8               # heads per core
SCALE = 1.0 / float(np.sqrt(np.float32(S)))

F32 = mybir.dt.float32
F32R = mybir.dt.float32r
BF16 = mybir.dt.bfloat16

S_CHUNK = 256                 # s columns of X^T staged per iteration
N_SCHUNKS = S // S_CHUNK      # 8
ST_PER_CHUNK = S_CHUNK // 128 # 2


def build_nc():
    nc = bacc.Bacc("TRN2", target_bir_lowering=False, debug=False)

    # --- per-core DRAM I/O (float32r is bit-identical to f32; enables
    # 1-cycle/row matmuls at free dim >= 256) ---
    xbt = nc.dram_tensor("xbt", [D, S], BF16, kind="ExternalInput")        # X[b].T
    wqkt = nc.dram_tensor("wqkt", [D, 1024], BF16, kind="ExternalInput")
    wvt = nc.dram_tensor("wvt", [D, 512], BF16, kind="ExternalInput")
    bqk = nc.dram_tensor("bqk", [1, 1024], F32, kind="ExternalInput")
    bvt = nc.dram_tensor("bvt", [128, 4], F32, kind="ExternalInput")       # bv.reshape(4,128).T
    w2t = nc.dram_tensor("w2t", [D, 1024], F32R, kind="ExternalInput")     # W2.T
    b2 = nc.dram_tensor("b2", [1, 1024], F32, kind="ExternalInput")
    out = nc.dram_tensor("out", [1024, 1024], F32, kind="ExternalOutput")

    # DRAM views with the 128-partition dim innermost-major for SBUF loads
    xbt_v = xbt[:].rearrange("(kb p) s -> p kb s", p=128)      # [128, 8, 2048]
    wvt_v = wvt[:].rearrange("(kb p) n -> p kb n", p=128)      # [128, 8, 512]
    w2t_v = w2t[:].rearrange("(jb p) n -> p jb n", p=128)      # [128, 8, 1024]
    # output rows r = 256*p + 128*g + 2*d + half
    out_v = out[:].rearrange("(p g d h) n -> p g d h n", p=4, g=2, d=64, h=2)

    with tile.TileContext(nc) as tc:
        with (
            tc.tile_pool(name="const", bufs=1) as const,
            tc.tile_pool(name="xin", bufs=3) as xin,
            tc.tile_pool(name="ywork", bufs=3) as ywork,
            tc.tile_pool(name="vtp", bufs=1) as vtp,
            tc.tile_pool(name="attn", bufs=1) as attn,
            tc.tile_pool(name="otp", bufs=2) as otp,
            tc.tile_pool(name="fout", bufs=2) as fout,
            tc.tile_pool(name="psacc", bufs=3, space="PSUM") as psacc,
            tc.tile_pool(name="pswt", bufs=1, space="PSUM") as pswt,
            tc.tile_pool(name="psot", bufs=2, space="PSUM") as psot,
            tc.tile_pool(name="psf", bufs=2, space="PSUM") as psf,
        ):
            # ---------------- phase-1 loads (phase-3 loads deferred) ------
            # The first QK matmul needs xbt chunk0 + wqk kb=0 only: submit
            # those first so the saturable (~25GB/s/engine) DMA engines serve
            # the critical path before the bulk weight traffic.
            xbt_tiles = []
            xbt_sb0 = xin.tile([128, 8, S_CHUNK], BF16, tag="xbt")
            nc.sync.dma_start(out=xbt_sb0[:], in_=xbt_v[:, :, 0:S_CHUNK])
            xbt_tiles.append(xbt_sb0)

            # Per-kb loads keep 4KB contiguous bursts and let the first QK
            # matmul start after only the kb=0 block (512KB) has landed.
            wqk_sb = const.tile([128, 8, 1024], BF16)
            for kb in range(8):
                nc.scalar.dma_start(out=wqk_sb[:, kb, :],
                                    in_=wqkt[kb * 128:(kb + 1) * 128, :])
            wv_sb = const.tile([128, 8, 512], BF16)
            for kb in range(0, 8, 2):
                nc.scalar.dma_start(out=wv_sb[:, kb:kb + 2, :],
                                    in_=wvt_v[:, kb:kb + 2, :])

            bqk_bc = const.tile([128, 1024], F32)
            nc.gpsimd.dma_start(out=bqk_bc[:], in_=bqk[:].to_broadcast((128, 1024)))
            bv_sb = const.tile([128, 4], F32)
            nc.gpsimd.dma_start(out=bv_sb[:], in_=bvt[:])

            ones_sb = const.tile([128, 1], F32)
            nc.vector.memset(ones_sb[:], 1.0)

            # VT persists until the OT matmuls; wT accumulates across all s.
            vt_sb = vtp.tile([128, 4, S], BF16)         # [e2, pair, s]
            psum_wt = pswt.tile([128, 512], F32)        # [e2(g*64+e), pair*128+c2]

            # ---------------- phase 1: QK, VT, wT ----------------
            for sc in range(N_SCHUNKS):
                if sc < len(xbt_tiles):
                    xbt_sb = xbt_tiles[sc]
                else:
                    xbt_sb = xin.tile([128, 8, S_CHUNK], BF16, tag="xbt")
                    nc.sync.dma_start(
                        out=xbt_sb[:],
                        in_=xbt_v[:, :, sc * S_CHUNK:(sc + 1) * S_CHUNK],
                    )

                yqk_tiles = []
                for st in range(ST_PER_CHUNK):
                    s_lo = st * 128
                    yqk_sb = ywork.tile([128, 1024], F32, tag="yqk")
                    yqk_tiles.append(yqk_sb)
                    for nh in range(2):
                        ps_y = psacc.tile([128, 512], F32, tag="acc")
                        for kb in range(8):
                            nc.tensor.matmul(
                                ps_y[:],
                                lhsT=xbt_sb[:, kb, s_lo:s_lo + 128],
                                rhs=wqk_sb[:, kb, nh * 512:(nh + 1) * 512],
                                start=(kb == 0),
                                stop=(kb == 7),
                            )
                        nc.vector.tensor_tensor(
                            out=yqk_sb[:, nh * 512:(nh + 1) * 512],
                            in0=ps_y[:],
                            in1=bqk_bc[:, nh * 512:(nh + 1) * 512],
                            op=mybir.AluOpType.add,
                        )

                # VT: out [vrow-block, s-chunk].  Traced between the QK
                # matmuls and the wT matmuls so the DVE evictions of yqk
                # have a full VT window to drain before PE needs them
                # (avoids head-of-line stalls on the in-order PE queue).
                for mb in range(4):
                    ps_v = psacc.tile([128, S_CHUNK], F32, tag="acc")
                    for kb in range(8):
                        nc.tensor.matmul(
                            ps_v[:],
                            lhsT=wv_sb[:, kb, mb * 128:(mb + 1) * 128],
                            rhs=xbt_sb[:, kb, :],
                            start=(kb == 0),
                            stop=(kb == 7),
                        )
                    nc.vector.tensor_scalar_add(
                        vt_sb[:, mb, sc * S_CHUNK:(sc + 1) * S_CHUNK],
                        ps_v[:],
                        bv_sb[:, mb:mb + 1],
                    )

                # wT accumulation: lhsT=K_h slice, rhs=Q_h slice
                for st in range(ST_PER_CHUNK):
                    yqk_sb = yqk_tiles[st]
                    for hl in range(NH):
                        p, g = hl // 2, hl % 2
                        nc.tensor.matmul(
                            psum_wt[g * 64:(g + 1) * 64,
                                    p * 128 + g * 64:p * 128 + (g + 1) * 64],
                            lhsT=yqk_sb[:, 512 + hl * 64:512 + (hl + 1) * 64],
                            rhs=yqk_sb[:, hl * 64:(hl + 1) * 64],
                            # start=True clears has_written for the WHOLE bank
                            # row of the written partitions -> only the first
                            # matmul per partition-half may set it.
                            start=(sc == 0 and st == 0 and hl < 2),
                            stop=(sc == N_SCHUNKS - 1 and st == ST_PER_CHUNK - 1),
                            skip_group_check=True,
                        )

            # ---------------- phase-3 weights (overlap with phase 1) ------
            w2_sb = const.tile([128, 8, 1024], F32R)
            nc.scalar.dma_start(out=w2_sb[:], in_=w2t_v)
            b2_bc = const.tile([128, 1024], F32)
            nc.gpsimd.dma_start(out=b2_bc[:], in_=b2[:].to_broadcast((128, 1024)))

            # ---------------- phase 2: exp, Z ----------------
            expw_f32 = attn.tile([128, 4, 128], F32)
            nc.vector.memset(expw_f32[:], 0.0)
            for hl in range(NH):
                p, g = hl // 2, hl % 2
                nc.scalar.activation(
                    out=expw_f32[g * 64:(g + 1) * 64, p, g * 64:(g + 1) * 64],
                    in_=psum_wt[g * 64:(g + 1) * 64,
                                p * 128 + g * 64:p * 128 + (g + 1) * 64],
                    func=mybir.ActivationFunctionType.Exp,
                    scale=SCALE,
                )
            expw_sb = attn.tile([128, 4, 128], BF16)
            nc.vector.tensor_copy(expw_sb[:], expw_f32[:])
            ps_z = psacc.tile([128, 4], F32, tag="acc")
            rz_sb = attn.tile([128, 4], F32)
            for p in range(4):
                nc.tensor.matmul(
                    ps_z[:, p:p + 1],
                    lhsT=expw_f32[:, p, :],
                    rhs=ones_sb[:],
                    start=(p == 0),
                    stop=(p == 3),
                    skip_group_check=True,
                )
            nc.vector.reciprocal(rz_sb[:], ps_z[:])

            # ---------------- phase 3: OT, F, store ----------------
            for p in range(4):
                ot_sb = otp.tile([128, 16, 128], F32R, tag="ot")
                for sbq in range(4):
                    ps_ot = psot.tile([128, 4, 128], F32, tag="psot")
                    for i in range(4):
                        nc.tensor.matmul(
                            ps_ot[:, i, :],
                            lhsT=vt_sb[:, p, (sbq * 4 + i) * 128:(sbq * 4 + i + 1) * 128],
                            rhs=expw_sb[:, p, :],
                            start=(i == 0),
                            stop=(i == 3),
                            skip_group_check=True,
                        )
                    nc.scalar.copy(ot_sb[:, sbq * 4:(sbq + 1) * 4, :], ps_ot[:])
                for half in range(2):
                    f_sb = fout.tile([128, 1024], F32, tag="f")
                    for nh in range(2):
                        ps_f = psf.tile([128, 512], F32, tag="psf")
                        for sb8 in range(8):
                            nc.tensor.matmul(
                                ps_f[:],
                                lhsT=ot_sb[:, half * 8 + sb8, :],
                                rhs=w2_sb[:, sb8, nh * 512:(nh + 1) * 512],
                                start=(sb8 == 0),
                                stop=(sb8 == 7),
                            )
                        # F = psum * rZ (per partition) + b2
                        nc.vector.scalar_tensor_tensor(
                            out=f_sb[:, nh * 512:(nh + 1) * 512],
                            in0=ps_f[:],
                            scalar=rz_sb[:, p:p + 1],
                            in1=b2_bc[:, nh * 512:(nh + 1) * 512],
                            op0=mybir.AluOpType.mult,
                            op1=mybir.AluOpType.add,
                        )
                    nc.sync.dma_start(out=out_v[p, :, :, half, :], in_=f_sb[:])

    nc.finalize()
    return nc


_NC_CACHE = None


def _get_nc():
    global _NC_CACHE
    if _NC_CACHE is None:
        _NC_CACHE = build_nc()
    return _NC_CACHE


def _shard_inputs(X, W1, b1, W2, b2):
    X = np.asarray(X, np.float32)
    W1 = np.asarray(W1, np.float32)
    b1 = np.asarray(b1, np.float32)
    W2 = np.asarray(W2, np.float32)
    b2 = np.asarray(b2, np.float32)

    w2t = np.ascontiguousarray(W2.T)
    b2r = np.ascontiguousarray(b2.reshape(1, 1024))
    xbts = [np.ascontiguousarray(X[b].T).astype(ml_dtypes.bfloat16)
            for b in range(B)]

    per_hg = []
    for hg in range(2):
        heads = range(NH * hg, NH * hg + NH)
        qrows = np.concatenate(
            [np.arange(h * DH, (h + 1) * DH) for h in heads]
            + [D + np.arange(h * DH, (h + 1) * DH) for h in heads])
        vrows = np.concatenate(
            [2 * D + np.arange(h * DH, (h + 1) * DH) for h in heads])
        wqkt = np.ascontiguousarray(W1[qrows].T).astype(ml_dtypes.bfloat16)
        bqk = np.ascontiguousarray(b1[qrows].reshape(1, 1024))
        wvt = np.ascontiguousarray(W1[vrows].T).astype(ml_dtypes.bfloat16)
        bvt = np.ascontiguousarray(b1[vrows].reshape(4, 128).T)
        per_hg.append((wqkt, bqk, wvt, bvt))

    in_maps = []
    for c in range(8):
        b, hg = c // 2, c % 2
        wqkt, bqk, wvt, bvt = per_hg[hg]
        in_maps.append({
            "xbt": xbts[b], "wqkt": wqkt, "wvt": wvt, "bqk": bqk,
            "bvt": bvt, "w2t": w2t, "b2": b2r,
        })
    return in_maps


def run(X, W1, b1, W2, b2, **run_kwargs):
    """Returns (full_output, BassKernelResults)."""
    nc = _get_nc()
    in_maps = _shard_inputs(X, W1, b1, W2, b2)
    res = run_bass_kernel_spmd(nc, in_maps, core_ids=list(range(8)), **run_kwargs)
    full = np.empty((B, S, D), np.float32)
    for c in range(8):
        b, hg = c // 2, c % 2
        full[b, hg * 1024:(hg + 1) * 1024, :] = res.results[c]["out"]
    return full, res


def kernel(X, W1, b1, W2, b2):
    return run(X, W1, b1, W2, b2)[0]
